# revision 7
# baseline (speedup 1.0000x reference)
"""Bass/Tile kernel for nn_Attention_49959059587521 on 8 TRN2 NeuronCores.

Math per (batch b, head h), with Q,K,V,Q2,K2 = [2048, 64] slices:
    S    = (Q @ K^T) * 0.125                    # [2048, 2048]
    P    = softmax(S, axis=-1)
    gate = sigmoid((Q2 @ sum_n(K2)) * 0.125)    # [2048]
    out  = (P * gate[:, None]) @ V              # [2048, 64]

Sharding: 32 (b, h) pairs over 8 cores -> core i handles b = i//2 and the 4
heads h in [4*(i%2), 4*(i%2)+4), i.e. the channel slice [256*(i%2), +256).
No cross-core communication.

Per-core algorithm (fully on device):
  - S^T[k, q] = K^T(stationary) x Q^T(moving) via float32r matmuls (1 cyc/row).
  - exp fused on ScalarE reading PSUM directly (scale=0.125 via free affine);
    no max-subtraction needed: logits are ~N(0,1), |S| < ~7, exp is safe in f32.
  - O^T = V'^T @ E accumulated in PSUM over the 16 k-tiles, where V' = [V; ones]
    so row 64 of O^T is the softmax denominator (free rowsum).
  - gate computed as 1/(1+exp(-z*scale)) (exp table only; inf-safe).
  - O^T 128-column blocks PE-transposed back to [q, d]; the PSUM->SBUF copy
    fuses the (gate * 1/rowsum) per-row scale on the VectorE.
"""

import functools
from contextlib import ExitStack

import numpy as np

import concourse.bass as bass
import concourse.mybir as mybir
import concourse.tile as tile
from concourse import bacc, bass_isa, bass_utils
from concourse.masks import make_identity

F32 = mybir.dt.float32
F32R = mybir.dt.float32r

B, NT, C, H = 4, 2048, 512, 8
HD = 64
SCALE = HD ** -0.5  # 0.125
P = 128
NO = NT // P            # 16 n-tiles
NH4 = 4                 # heads per core
CW = NH4 * HD           # 256 channels per core
NHALF = 2               # q processed in two halves of 1024
QH = NT // NHALF        # 1024
E_DT = F32R             # dtype of exp(S) tiles fed to the second matmul


def _build(use_sigmoid: bool):
    nc = bacc.Bacc("TRN2", target_bir_lowering=False)
    q_d = nc.dram_tensor("q", [NT, CW], F32, kind="ExternalInput")
    k_d = nc.dram_tensor("k", [NT, CW], F32, kind="ExternalInput")
    v_d = nc.dram_tensor("v", [NT, CW], F32, kind="ExternalInput")
    if use_sigmoid:
        q2_d = nc.dram_tensor("q2", [NT, CW], F32, kind="ExternalInput")
        k2_d = nc.dram_tensor("k2", [NT, CW], F32, kind="ExternalInput")
    out_d = nc.dram_tensor("out", [NT, CW], F32, kind="ExternalOutput")

    with tile.TileContext(nc) as tc, ExitStack() as ctx:
        singles = ctx.enter_context(tc.tile_pool(name="singles", bufs=1))
        tpool = ctx.enter_context(tc.tile_pool(name="tp", bufs=2))
        epool = ctx.enter_context(tc.tile_pool(name="ep", bufs=3))
        opool = ctx.enter_context(tc.tile_pool(name="op", bufs=2))
        # PSUM: st 2x[128,1024] = 4 banks, acc 1x[65,1024] = 2 banks,
        # tr 2x[<=128,<=512] = 2 banks. Total 8 banks.
        ps_st = ctx.enter_context(tc.tile_pool(name="ps_st", bufs=2, space="PSUM"))
        ps_ac = ctx.enter_context(tc.tile_pool(name="ps_ac", bufs=1, space="PSUM"))
        ps_tr = ctx.enter_context(tc.tile_pool(name="ps_tr", bufs=2, space="PSUM"))

        def tr_tile(shape):
            return ps_tr.tile(shape, F32, tag="ptr", name="ptr")

        # ---- stage A: bulk input loads ([n, c] -> [p, o, c] tiling) ----
        def load_tiled(dram, nm):
            t = singles.tile([P, NO, CW], F32, name=nm, tag=nm)
            nc.sync.dma_start(t, dram.ap().rearrange("(o p) c -> p o c", p=P))
            return t

        q_sb = load_tiled(q_d, "q_sb")
        k_sb = load_tiled(k_d, "k_sb")

        # V' = [V | ones]: [p, o, head, 65]; col 64 = 1.0 (rowsum trick)
        v_st = singles.tile([P, NO, NH4, HD + 1], F32)
        v_ap4 = v_d.ap().rearrange("(o p) (j c) -> p o j c", p=P, j=NH4)
        for j in range(NH4):
            nc.sync.dma_start(v_st[:, :, j, 0:HD], v_ap4[:, :, j, :])
        nc.vector.memset(v_st[:, :, :, HD : HD + 1], 1.0)
        v1r = singles.tile([P, NO, NH4, HD + 1], F32R)
        nc.vector.tensor_copy(v1r, v_st)

        ident = singles.tile([P, P], F32)
        make_identity(nc, ident)

        # ---- stage C-pre: k2sum broadcast to all partitions (gate path) ----
        if use_sigmoid:
            q2_sb = load_tiled(q2_d, "q2_sb")
            k2_sb = load_tiled(k2_d, "k2_sb")
            k2o = singles.tile([P, CW], F32)
            nc.vector.reduce_sum(
                out=k2o,
                in_=k2_sb.rearrange("p o c -> p c o"),
                axis=mybir.AxisListType.X,
            )
            k2b_sb = singles.tile([P, CW], F32)
            nc.gpsimd.partition_all_reduce(
                k2b_sb, k2o, channels=P, reduce_op=bass_isa.ReduceOp.add
            )

        out_ap3 = out_d.ap().rearrange("(o p) c -> p o c", p=P)

        for j in range(NH4):  # local head
            ch = HD * j
            # ---- stage B: per-head transposes q, k -> [64, 2048] f32r ----
            qT = tpool.tile([HD, NT], F32R, tag="qT")
            kT = tpool.tile([HD, NT], F32R, tag="kT")
            for src, dst in ((q_sb, qT), (k_sb, kT)):
                for g in range(NO // 4):
                    tp = tr_tile([HD, 4 * P])
                    for u in range(4):
                        o = 4 * g + u
                        nc.tensor.transpose(
                            tp[:, P * u : P * (u + 1)],
                            src[:, o, ch : ch + HD],
                            ident,
                        )
                    nc.vector.tensor_copy(dst[:, 4 * P * g : 4 * P * (g + 1)], tp)

            # ---- stage C: gate g = sigmoid(scale * q2 . k2sum) as [128, 16] ----
            if use_sigmoid:
                zt = opool.tile([P, NO, HD], F32, tag="zt")
                nc.vector.tensor_mul(
                    zt,
                    q2_sb[:, :, ch : ch + HD],
                    k2b_sb[:, None, ch : ch + HD].to_broadcast((P, NO, HD)),
                )
                z = opool.tile([P, NO], F32, tag="z")
                nc.vector.reduce_sum(out=z, in_=zt, axis=mybir.AxisListType.X)
                eg = opool.tile([P, NO], F32, tag="eg")
                nc.scalar.activation(
                    eg, z, mybir.ActivationFunctionType.Exp, scale=-SCALE
                )
                nc.vector.tensor_scalar_add(eg, eg, 1.0)
                gte = opool.tile([P, NO], F32, tag="gte")
                nc.vector.reciprocal(gte, eg)

            obuf = opool.tile([P, NO, HD], F32, tag="obuf")

            for h in range(NHALF):  # q half
                q0 = QH * h
                # ---- stage D: S^T -> exp -> O^T accumulation ----
                acc = ps_ac.tile([HD + 1, QH], F32, tag="pac")
                for t in range(NO):
                    st = ps_st.tile([P, QH], F32, tag="pst")
                    for s2 in range(QH // 512):
                        nc.tensor.matmul(
                            st[:, 512 * s2 : 512 * (s2 + 1)],
                            kT[:, P * t : P * (t + 1)],
                            qT[:, q0 + 512 * s2 : q0 + 512 * (s2 + 1)],
                            start=True,
                            stop=True,
                        )
                    et = epool.tile([P, QH], E_DT, tag="et")
                    nc.scalar.activation(
                        et, st, mybir.ActivationFunctionType.Exp, scale=SCALE
                    )
                    for s2 in range(QH // 512):
                        nc.tensor.matmul(
                            acc[:, 512 * s2 : 512 * (s2 + 1)],
                            v1r[:, t, j, :],
                            et[:, 512 * s2 : 512 * (s2 + 1)],
                            start=(t == 0),
                            stop=(t == NO - 1),
                        )
                ot_sb = opool.tile([HD + 1, QH], F32, tag="ot")
                nc.vector.tensor_copy(ot_sb, acc)

                # ---- phase 2: transpose back + normalize (+gate) ----
                for u in range(QH // P):
                    i = (QH // P) * h + u
                    tr = tr_tile([P, HD + 1])
                    nc.tensor.transpose(
                        tr,
                        ot_sb[:, P * u : P * (u + 1)],
                        ident[: HD + 1, : HD + 1],
                    )
                    rcp = opool.tile([P, 1], F32, tag="rcp")
                    nc.vector.reciprocal(rcp, tr[:, HD : HD + 1])
                    if use_sigmoid:
                        fac = opool.tile([P, 1], F32, tag="fac")
                        nc.vector.tensor_mul(fac, rcp, gte[:, i : i + 1])
                    else:
                        fac = rcp
                    nc.vector.tensor_scalar_mul(obuf[:, i, :], tr[:, 0:HD], fac)

            nc.sync.dma_start(out_ap3[:, :, ch : ch + HD], obuf)

    nc.compile()
    return nc


@functools.lru_cache(maxsize=2)
def _graph(use_sigmoid: bool):
    return _build(use_sigmoid)


def _shard(a: np.ndarray, i: int) -> np.ndarray:
    b, hg = divmod(i, 2)
    return np.ascontiguousarray(a[b, :, hg * CW : (hg + 1) * CW], dtype=np.float32)


def run(inputs, trace: bool = False):
    use_sigmoid = bool(np.asarray(inputs["use_sigmoid"]).item())
    nc = _graph(use_sigmoid)
    in_maps = []
    for i in range(8):
        m = {
            "q": _shard(np.asarray(inputs["query"]), i),
            "k": _shard(np.asarray(inputs["key"]), i),
            "v": _shard(np.asarray(inputs["value"]), i),
        }
        if use_sigmoid:
            m["q2"] = _shard(np.asarray(inputs["query2"]), i)
            m["k2"] = _shard(np.asarray(inputs["key2"]), i)
        in_maps.append(m)
    res = bass_utils.run_bass_kernel_spmd(
        nc, in_maps, core_ids=list(range(8)), trace=trace
    )
    out = np.empty((B, NT, C), dtype=np.float32)
    for i in range(8):
        b, hg = divmod(i, 2)
        out[b, :, hg * CW : (hg + 1) * CW] = res.results[i]["out"]
    return out, res


def kernel(**inputs) -> np.ndarray:
    out, _ = run(inputs)
    return out


if __name__ == "__main__":
    rng = np.random.default_rng(0)
    fake = {
        "query": rng.standard_normal((B, NT, C), dtype=np.float32),
        "key": rng.standard_normal((B, NT, C), dtype=np.float32),
        "value": rng.standard_normal((B, NT, C), dtype=np.float32),
        "query2": rng.standard_normal((B, NT, C), dtype=np.float32),
        "key2": rng.standard_normal((B, NT, C), dtype=np.float32),
        "use_sigmoid": 1,
    }
    out = kernel(**fake)
    print("ran ok", out.shape, out.dtype)
